# revision 36
# baseline (speedup 1.0000x reference)
"""Trainium2 Bass kernel for the eigenvalue/eigenvector loss
(nn_AV_loss): per-voxel 3x3 symmetric eigendecomposition of input and
target tensors, masked L1 of sorted eigenvalues + masked principal-axis
|cosine|, reduced to two scalars.

Self-contained: hardcodes shapes/sharding. kernel(**inputs) takes FULL
inputs and returns the full output (val_loss, vec_loss).

Sharding: fully data-parallel over B*H (2*80 = 160 -> 20 H-slices per
core); per-core partial masked sums are returned and reduced on host.

Math (per 3x3 symmetric matrix A = [[a,d,e],[d,b,f],[e,f,c]]):
  trigonometric (Smith) eigensolver:
    q = tr/3, p = sqrt(p2/6) with p2 = sum of squared deviator entries,
    r = det(A-qI)/(2 p^3) clamped to [-1,1];
    half-angle arctan keeps the ACT input in [-1, 1]:
      acos(r)/2 = pi/4 + atan((sm-sp)/(sm+sp)),
      sp = sqrt((1+r)/2), sm = sqrt((1-r)/2)
    lam_max = q + 2p*sin(pi/3 - 2at/3), lam_min = q - 2p*sin(pi/3 + 2at/3),
    lam_mid = 3q - lam_max - lam_min.
  principal eigenvector via cross product of the first two rows of
  (A - lam_max I)  (parallel-rows failure set has measure ~0 and its
  bounded error washes out in the 512k-voxel masked mean).

Precision: inputs are converted to bf16 on the host (halves DMA bytes);
the elementwise pipeline runs bf16 on the DVE (2x/4x perf modes) with
f32 for the reciprocal-seeded chains and the final accumulations.
Validated end-to-end error vs the fp64 reference is ~2e-4 relative.

Masked-voxel compaction: voxel -> (partition, column) placement is the
host's choice, so the host packs ALL masked voxels of a core row-major
into [128, CW] (CW=512, ~2% padding with a benign diag(1,2,3) matrix
that adds exactly 0 to the eigenvalue-L1 sum and exactly 1 per pad to
the |dot| sum, subtracted on host). The device never sees unmasked
voxels and no mask multiply exists on device.

Engine split: DVE tensor-tensor chains; ACT all squares (with free
scale folding: Square(sqrt(2)*x) = 2x^2), sqrt/arctan/sin/abs; GPSIMD
leaf products (d*e, d*f). ACT table-set phases (sqrt -> trig -> sqrt)
are enforced with explicit ordering edges so the scheduler cannot
thrash table loads. Measured: ~71 us on 8 cores, rel err ~2.8e-4.
"""

import numpy as np
import ml_dtypes

import concourse.tile as tile
from concourse import mybir
from concourse.bacc import Bacc
from concourse.bass_utils import run_bass_kernel_spmd
from bass_rust import add_dep_helper


class _CapacityError(RuntimeError):
    pass


AF = mybir.ActivationFunctionType
OP = mybir.AluOpType
F32 = mybir.dt.float32
BF16 = mybir.dt.bfloat16
U16 = mybir.dt.uint16

NCORES = 8
B, C, H, W, D = 2, 6, 80, 80, 80
HS = H // (NCORES // B)          # 20 h-slices per core
VPC = HS * W * D                 # 128000 voxels per core
P = 128
FV = VPC // P                    # 1000 voxel columns per partition
NCH = 1                          # chunks along the free dim
FC = FV // NCH                   # raw voxel cols per chunk (500)
NPAD = 8                         # benign pad columns appended per plane
FCP = 2 * FC + NPAD              # raw plane cols: [input | target | pad]
CW = 512                         # compact width (global row-major fill)
PK = 2 * CW                      # compact packed cols: [input | target]

# benign pad matrix diag(1,2,3): lam={3,2,1}, input==target so d|lam|=0
# and |cos|=1 exactly per pad (host subtracts the pad count)
PAD_CH = {"a": 1.0, "b": 2.0, "c": 3.0, "d": 0.0, "e": 0.0, "f": 0.0}
CH_ORDER = (("d", 1), ("e", 2), ("f", 4), ("a", 0), ("b", 3), ("c", 5))

CLAMP = 1.0 - 3e-7
PI3 = float(np.pi / 3.0)
SQRT2 = float(np.sqrt(2.0))


def _build():
    nc = Bacc()
    x = nc.dram_tensor("x", [C, P, NCH, PK], BF16, kind="ExternalInput")
    out = nc.dram_tensor("out", [P, 2 * NCH], F32, kind="ExternalOutput")

    with tile.TileContext(nc) as tc:
        with tc.tile_pool(name="main", bufs=1) as pool:

            def T(tag, cols=PK, dt=BF16):     # per-chunk persisted value
                return pool.tile([P, cols], dt, tag=tag, bufs=NCH, name=tag)

            def TA():                          # phase-A bf16 transient
                return pool.tile([P, PK], BF16, tag="tA", bufs=16, name="tA")

            def TA32():                        # phase-A f32 transient
                return pool.tile([P, PK], F32, tag="tA32", bufs=4, name="tA32")

            def TB():                          # phase-B bf16 transient
                return pool.tile([P, PK], BF16, tag="tB", bufs=14, name="tB")

            def THB():                         # half-width bf16 transient
                return pool.tile([P, CW], BF16, tag="tHB", bufs=12, name="tHB")

            def TH32():                        # half-width f32 transient
                return pool.tile([P, CW], F32, tag="tH32", bufs=6, name="tH32")

            out_sb = pool.tile([P, 2 * NCH], F32, tag="out_sb", bufs=1)
            c05 = pool.tile([P, 1], F32, tag="c05", bufs=1)
            nc.vector.memset(c05, 0.5)
            pi3c = pool.tile([P, 1], F32, tag="pi3c", bufs=1)
            nc.vector.memset(pi3c, PI3)
            warm = pool.tile([P, 1], F32, tag="warm", bufs=1)
            nc.scalar.activation(out=warm, in_=c05, func=AF.Sqrt)

            # ---- loads (host already compacted masked voxels) ----
            chans = []
            for cidx in range(NCH):
                cd = {}
                for nm, ch in CH_ORDER:
                    tl = T("ch_" + nm)
                    nc.sync.dma_start(out=tl, in_=x[ch, :, cidx, :])
                    cd[nm] = tl
                chans.append(cd)

            acts_A = []   # ACT instructions per phase, for ordering edges
            acts_B = []
            acts_C = []

            # GPSIMD leaf products (only need channel DMAs) hoisted early so
            # the GpSimd engine works during phase A instead of idling
            leaves = []
            for cidx in range(NCH):
                ch = chans[cidx]
                de = pool.tile([P, PK], BF16, tag="de_lf", bufs=NCH, name="de_lf")
                nc.gpsimd.tensor_tensor(out=de, in0=ch["d"], in1=ch["e"],
                                        op=OP.mult)
                m1 = pool.tile([P, PK], BF16, tag="m1_lf", bufs=NCH, name="m1_lf")
                nc.gpsimd.tensor_tensor(out=m1, in0=ch["d"], in1=ch["f"],
                                        op=OP.mult)
                leaves.append((de, m1))

            # ---- phase A (sqrt act-set): invariants, p, r, atan arg ----
            pers = []
            for cidx in range(NCH):
                ch = chans[cidx]
                a, b, c = ch["a"], ch["b"], ch["c"]
                d, e, f = ch["d"], ch["e"], ch["f"]

                sAB = TA()
                nc.vector.tensor_add(out=sAB, in0=a, in1=b)
                s3 = T("s3")
                nc.vector.tensor_add(out=s3, in0=sAB, in1=c)
                q = T("q")
                nc.vector.tensor_scalar_mul(out=q, in0=s3, scalar1=1.0 / 3.0)
                aq = TA()
                nc.vector.tensor_sub(out=aq, in0=a, in1=q)
                bq = TA()
                nc.vector.tensor_sub(out=bq, in0=b, in1=q)
                cq = TA()
                nc.vector.tensor_sub(out=cq, in0=c, in1=q)
                # pre-doubled squares: Square(sqrt(2)x) = 2x^2 (free scale)
                dd2 = T("dd2")
                acts_A.append(nc.scalar.activation(
                    out=dd2, in_=d, func=AF.Square, scale=SQRT2).ins)
                ee2 = TA()
                acts_A.append(nc.scalar.activation(
                    out=ee2, in_=e, func=AF.Square, scale=SQRT2).ins)
                ff2 = TA()
                acts_A.append(nc.scalar.activation(
                    out=ff2, in_=f, func=AF.Square, scale=SQRT2).ins)
                aq2 = TA()
                acts_A.append(nc.scalar.activation(
                    out=aq2, in_=aq, func=AF.Square).ins)
                bq2 = TA()
                acts_A.append(nc.scalar.activation(
                    out=bq2, in_=bq, func=AF.Square).ins)
                cq2 = TA()
                acts_A.append(nc.scalar.activation(
                    out=cq2, in_=cq, func=AF.Square).ins)
                p12 = TA()
                nc.vector.tensor_add(out=p12, in0=dd2, in1=ee2)
                p12b = TA()
                nc.vector.tensor_add(out=p12b, in0=p12, in1=ff2)
                t = TA()
                nc.vector.tensor_add(out=t, in0=aq2, in1=bq2)
                t2 = TA()
                nc.vector.tensor_add(out=t2, in0=t, in1=cq2)
                p2 = TA()
                nc.vector.tensor_add(out=p2, in0=t2, in1=p12b)
                # tp = 2p = sqrt(p2 * 2/3)
                tp = T("tp")
                acts_A.append(nc.scalar.activation(
                    out=tp, in_=p2, func=AF.Sqrt, scale=2.0 / 3.0).ins)
                tpsq = TA()
                acts_A.append(nc.scalar.activation(
                    out=tpsq, in_=tp, func=AF.Square).ins)       # 4p^2
                p3x = TA32()
                nc.vector.tensor_mul(out=p3x, in0=tpsq, in1=tp)  # 8p^3
                ip8 = TA32()
                nc.vector.reciprocal_approx_fast(out=ip8, in_=p3x)

                # 2*det = abc2 + def4 - aff - bee - cdd  (all pre-doubled)
                de = leaves[cidx][0]
                f4 = TA()
                nc.vector.tensor_scalar_mul(out=f4, in0=f, scalar1=4.0)
                def4 = TA()
                nc.vector.tensor_mul(out=def4, in0=de, in1=f4)
                cq2x = TA()
                nc.vector.tensor_scalar_mul(out=cq2x, in0=cq, scalar1=2.0)
                bc2 = TA()
                nc.vector.tensor_mul(out=bc2, in0=bq, in1=cq2x)
                abc2 = TA()
                nc.vector.tensor_mul(out=abc2, in0=aq, in1=bc2)
                aff = TA()
                nc.vector.tensor_mul(out=aff, in0=aq, in1=ff2)
                bee = TA()
                nc.vector.tensor_mul(out=bee, in0=bq, in1=ee2)
                cdd = TA()
                nc.vector.tensor_mul(out=cdd, in0=cq, in1=dd2)
                s1 = TA()
                nc.vector.tensor_add(out=s1, in0=abc2, in1=def4)
                s2d = TA()
                nc.vector.tensor_add(out=s2d, in0=aff, in1=bee)
                s3d = TA()
                nc.vector.tensor_add(out=s3d, in0=s2d, in1=cdd)
                D2 = TA()
                nc.vector.tensor_sub(out=D2, in0=s1, in1=s3d)

                # r = det/(2p^3) = (D2 * 2) * ip8, clamped to +-CLAMP
                r0 = TA()
                nc.vector.scalar_tensor_tensor(out=r0, in0=D2, scalar=2.0,
                                               in1=ip8, op0=OP.mult,
                                               op1=OP.mult)
                r = TA()
                nc.vector.tensor_scalar(out=r, in0=r0, scalar1=CLAMP,
                                        scalar2=-CLAMP, op0=OP.min, op1=OP.max)
                # half-width pipelined atan section: ACT works on one
                # half while the DVE processes the other, hiding the
                # serial ACT<->DVE ping-pong latency
                sp = TA()
                sm = TA()
                num = TA()
                den = TA32()
                iden = TA32()
                arg = T("arg")
                for h in range(2):
                    hs = slice(h * CW, (h + 1) * CW)
                    acts_A.append(nc.scalar.activation(
                        out=sp[:, hs], in_=r[:, hs], func=AF.Sqrt,
                        scale=0.5, bias=c05).ins)
                    acts_A.append(nc.scalar.activation(
                        out=sm[:, hs], in_=r[:, hs], func=AF.Sqrt,
                        scale=-0.5, bias=c05).ins)
                    nc.vector.tensor_sub(out=num[:, hs], in0=sm[:, hs],
                                         in1=sp[:, hs])
                    nc.vector.tensor_add(out=den[:, hs], in0=sm[:, hs],
                                         in1=sp[:, hs])
                    nc.vector.reciprocal_approx_fast(out=iden[:, hs],
                                                     in_=den[:, hs])
                    nc.vector.tensor_mul(out=arg[:, hs], in0=num[:, hs],
                                         in1=iden[:, hs])
                pers.append(dict(s3=s3, q=q, tp=tp, arg=arg, dd2=dd2))

            # ---- phase B (trig act-set) ----
            trig = []
            for cidx in range(NCH):
                arg = pers[cidx]["arg"]
                at = pool.tile([P, PK], BF16, tag="at_t", bufs=NCH,
                               name="at_t")
                c1 = pool.tile([P, PK], BF16, tag="c1_t", bufs=NCH,
                               name="c1_t")
                nc3 = pool.tile([P, PK], BF16, tag="nc3_t", bufs=NCH,
                               name="nc3_t")
                for h in range(2):
                    hs = slice(h * CW, (h + 1) * CW)
                    acts_B.append(nc.scalar.activation(
                        out=at[:, hs], in_=arg[:, hs], func=AF.Arctan).ins)
                    acts_B.append(nc.scalar.activation(
                        out=c1[:, hs], in_=at[:, hs], func=AF.Sin,
                        scale=-2.0 / 3.0, bias=pi3c).ins)
                    acts_B.append(nc.scalar.activation(
                        out=nc3[:, hs], in_=at[:, hs], func=AF.Sin,
                        scale=2.0 / 3.0, bias=pi3c).ins)
                trig.append((c1, nc3))

            persB = []
            for cidx in range(NCH):
                ch = chans[cidx]
                pr = pers[cidx]
                a, b, d, e, f = ch["a"], ch["b"], ch["d"], ch["e"], ch["f"]
                s3, q, tp = pr["s3"], pr["q"], pr["tp"]
                dd2 = pr["dd2"]
                de, m1 = leaves[cidx]
                c1, nc3 = trig[cidx]

                pc1 = TB()
                nc.vector.tensor_mul(out=pc1, in0=tp, in1=c1)
                l1 = TB()
                nc.vector.tensor_add(out=l1, in0=pc1, in1=q)    # lam_max
                pc3 = TB()
                nc.vector.tensor_mul(out=pc3, in0=tp, in1=nc3)
                l3 = TB()
                nc.vector.tensor_sub(out=l3, in0=q, in1=pc3)    # lam_min
                sl = TB()
                nc.vector.tensor_add(out=sl, in0=l1, in1=l3)
                l2 = TB()
                nc.vector.tensor_sub(out=l2, in0=s3, in1=sl)    # lam_mid

                # eigvec: cross(rows 0,1) of (A - l1*I)
                a1 = TB()
                nc.vector.tensor_sub(out=a1, in0=a, in1=l1)
                b1 = TB()
                nc.vector.tensor_sub(out=b1, in0=b, in1=l1)
                m2 = TB()
                nc.vector.tensor_mul(out=m2, in0=e, in1=b1)
                w1 = TB()
                nc.vector.tensor_sub(out=w1, in0=m1, in1=m2)
                m4 = TB()
                nc.vector.tensor_mul(out=m4, in0=a1, in1=f)
                w2 = TB()
                nc.vector.tensor_sub(out=w2, in0=de, in1=m4)
                m5 = TB()
                nc.vector.tensor_mul(out=m5, in0=a1, in1=b1)
                dd05 = TB()
                nc.vector.tensor_scalar_mul(out=dd05, in0=dd2, scalar1=0.5)
                w3 = TB()
                nc.vector.tensor_sub(out=w3, in0=m5, in1=dd05)

                sw1 = TB()
                acts_B.append(nc.scalar.activation(
                    out=sw1, in_=w1, func=AF.Square).ins)
                sw2 = TB()
                acts_B.append(nc.scalar.activation(
                    out=sw2, in_=w2, func=AF.Square).ins)
                sw3 = TB()
                acts_B.append(nc.scalar.activation(
                    out=sw3, in_=w3, func=AF.Square).ins)
                n12 = TB()
                nc.vector.tensor_add(out=n12, in0=sw1, in1=sw2)
                nrm = TB()
                nc.vector.tensor_add(out=nrm, in0=n12, in1=sw3)

                def IH(tl):
                    return tl[:, 0:CW]

                def THF(tl):
                    return tl[:, CW:PK]

                last = cidx == NCH - 1

                def TT2(out, in0, in1, op, gps=None):
                    use_gps = (not last) if gps is None else gps
                    if use_gps:
                        nc.gpsimd.tensor_tensor(out=out, in0=in0, in1=in1,
                                                op=op)
                    else:
                        nc.vector.tensor_tensor(out=out, in0=in0, in1=in1,
                                                op=op)

                nn0 = TH32()
                TT2(nn0, IH(nrm), THF(nrm), OP.mult)
                nn = TH32()
                nc.vector.tensor_scalar_add(out=nn, in0=nn0, scalar1=1e-30)
                inn = T("inn", cols=CW, dt=F32)
                nc.vector.reciprocal_approx_fast(out=inn, in_=nn)
                d1 = THB()
                TT2(d1, IH(w1), THF(w1), OP.mult)
                d2 = THB()
                TT2(d2, IH(w2), THF(w2), OP.mult)
                d3 = THB()
                TT2(d3, IH(w3), THF(w3), OP.mult)
                d12 = THB()
                TT2(d12, d1, d2, OP.add)
                dotv = THB()
                TT2(dotv, d12, d3, OP.add)
                adot = T("adot", cols=CW)
                acts_B.append(nc.scalar.activation(
                    out=adot, in_=dotv, func=AF.Abs).ins)

                dl = pool.tile([P, CW, 3], BF16, tag="dl", bufs=NCH,
                               name="dl")
                TT2(dl[:, :, 0], IH(l1), THF(l1), OP.subtract)
                TT2(dl[:, :, 1], IH(l2), THF(l2), OP.subtract)
                TT2(dl[:, :, 2], IH(l3), THF(l3), OP.subtract)
                persB.append(dict(inn=inn, adot=adot, dl=dl))

            # ---- phase C (sqrt act-set): normalize + masked reductions ----
            for cidx in range(NCH):
                pb = persB[cidx]
                rn = TH32()
                acts_C.append(nc.scalar.activation(
                    out=rn, in_=pb["inn"], func=AF.Sqrt).ins)
                nc.vector.tensor_reduce(out=out_sb[:, 2 * cidx:2 * cidx + 1],
                                        in_=pb["dl"],
                                        axis=mybir.AxisListType.XY,
                                        op=OP.add, apply_absolute_value=True)
                junk = TH32()
                nc.vector.scalar_tensor_tensor(
                    out=junk, in0=pb["adot"], scalar=1.0, in1=rn,
                    op0=OP.mult, op1=OP.mult,
                    accum_out=out_sb[:, 2 * cidx + 1:2 * cidx + 2])

            nc.sync.dma_start(out=out[:, :], in_=out_sb)

            # ACT phase-ordering edges: all sqrt-set ops before any trig-set
            # op, all trig-set ops before the final sqrt-set ops. Ordering
            # edges only (same engine), so no extra semaphores.
            for later in acts_B:
                add_dep_helper(later, acts_A[-1], False,
                               "act table phase order A->B")
            for later in acts_C:
                add_dep_helper(later, acts_B[-1], False,
                               "act table phase order B->C")
    nc.finalize()
    return nc


_NC = None


def _get_nc():
    global _NC
    if _NC is None:
        _NC = _build()
    return _NC


def _shard_inputs(input_data, target, mask):
    """Full inputs -> per-core in_maps: bf16 packed channel planes with
    benign pad columns, plus per-row compaction gather indices."""
    x = np.asarray(input_data, dtype=np.float32)
    t = np.asarray(target, dtype=np.float32)
    m = np.asarray(mask)
    in_maps = []
    total_pads = 0
    for k in range(NCORES):
        bidx = k // (NCORES // B)
        h0 = HS * (k % (NCORES // B))
        xs = x[bidx, :, h0:h0 + HS].reshape(C, P, NCH, FC)
        ts = t[bidx, :, h0:h0 + HS].reshape(C, P, NCH, FC)
        pad = np.empty((C, P, NCH, NPAD), np.float32)
        for nm, ch in CH_ORDER:
            pad[ch] = PAD_CH[nm]
        xc = np.concatenate([xs, ts, pad], axis=-1)      # [C,P,NCH,FCP]

        mb = (m[bidx, 0, 0, h0:h0 + HS].reshape(P, NCH, FC) == 1)
        # global compaction per chunk: voxel->slot placement is free, so
        # pack ALL masked voxels of the chunk row-major into [P, CW]
        xg = np.empty((C, P, NCH, PK), np.float32)
        for cidx in range(NCH):
            flat = mb[:, cidx, :].reshape(-1)             # [P*FC]
            pos = np.flatnonzero(flat)                    # masked voxel ids
            ncnt = pos.size
            if ncnt > P * CW:
                raise _CapacityError(
                    f"masked count {ncnt} exceeds capacity {P * CW}")
            total_pads += P * CW - ncnt
            planes = xc[:, :, cidx, :]                    # [C,P,FCP]
            flat_in = planes[:, :, :FC].reshape(C, -1)    # [C, P*FC]
            flat_tg = planes[:, :, FC:2 * FC].reshape(C, -1)
            gin = np.empty((C, P * CW), np.float32)
            gtg = np.empty((C, P * CW), np.float32)
            gin[:, :ncnt] = flat_in[:, pos]
            gtg[:, :ncnt] = flat_tg[:, pos]
            for nm, ch in CH_ORDER:
                gin[ch, ncnt:] = PAD_CH[nm]
                gtg[ch, ncnt:] = PAD_CH[nm]
            xg[:, :, cidx, :CW] = gin.reshape(C, P, CW)
            xg[:, :, cidx, CW:] = gtg.reshape(C, P, CW)
        in_maps.append({
            "x": np.ascontiguousarray(xg.astype(ml_dtypes.bfloat16)),
        })
    return in_maps, total_pads


def _host_reference(input_data, target, mask):
    """Exact numpy fallback (only if a mask ever exceeds the compact
    capacity, which cannot happen for the advertised input statistics)."""
    idx = np.array([[0, 1, 2], [1, 3, 4], [2, 4, 5]])

    def sym(t):
        return np.moveaxis(t, 1, -1)[..., idx]

    m = (np.asarray(mask)[:, 0, 0] == 1)
    mf = m.astype(np.float64)
    cntv = mf.sum()
    wi, vi = np.linalg.eigh(sym(np.asarray(input_data, np.float64)))
    wt, vt = np.linalg.eigh(sym(np.asarray(target, np.float64)))
    val = (np.abs(wi - wt).sum(-1) * mf).sum() / (3.0 * cntv)
    dot = np.abs((vi[..., :, 2] * vt[..., :, 2]).sum(-1))
    vec = 1.0 - (dot * mf).sum() / cntv
    return (np.float32(val), np.float32(vec))


def kernel(input_data, target, mask, root_dir=0, _trace=False):
    nc = _get_nc()
    try:
        in_maps, total_pads = _shard_inputs(
            np.asarray(input_data), np.asarray(target), np.asarray(mask))
    except _CapacityError:
        return _host_reference(input_data, target, mask)
    res = run_bass_kernel_spmd(nc, in_maps, core_ids=list(range(NCORES)),
                               trace=_trace)
    outs = res.results
    val_sum = 0.0
    dot_sum = 0.0
    for om in outs:
        o = om["out"].astype(np.float64)
        val_sum += o[:, 0::2].sum()
        dot_sum += o[:, 1::2].sum()
    dot_sum -= total_pads          # each pad contributes exactly |cos| = 1
    cnt = float((np.asarray(mask)[:, 0, 0] == 1).sum())
    val_loss = np.float32(val_sum / (3.0 * cnt))
    vec_loss = np.float32(1.0 - dot_sum / cnt)
    if _trace:
        return (val_loss, vec_loss), res
    return (val_loss, vec_loss)
